# revision 44
# baseline (speedup 1.0000x reference)
"""CrossNet kernel for Trainium2 (Bass/Tile), data-parallel over 8 NeuronCores.

Reference computation (per layer l = 0..3):
    xw     = einsum('bd,d->b', x, w_l)
    x_next = x0 * xw[:, None] + b_l[None, :] + x

Algebraic restructure: every layer adds (x0 * scalar_per_row + const_row), so
    x_l = x0 * alpha_l[:, None] + y_l[None, :]
with
    t_l     = x0 @ w_l            (per-row scalars, all 4 from one thin matmul)
    c_l     = y_l . w_l           (host-computed layer constants)
    alpha_0 = 1,  alpha_{l+1} = alpha_l * (1 + t_l) + c_l
    y_0     = 0,  y_{l+1}     = y_l + b_l
The kernel computes t = x0 @ W^T on the PE (via on-chip PE transposes of x0
tiles), the tiny alpha recurrence on the DVE, and the final scale
out = x0 * alpha_L (+ y_L) as one elementwise pass. This is numerically
equivalent to the reference up to fp32 rounding (~3e-7 rel err) and makes the
problem HBM-bandwidth-bound (one read + one write of the activation tensor).

Sharding: batch dim split across 8 cores (2048 rows each); weights replicated.
"""

import numpy as np

import concourse.bass as bass
import concourse.mybir as mybir
import concourse.tile as tile
from concourse import bacc
from concourse.bass_utils import run_bass_kernel_spmd
from concourse.masks import make_identity

N_CORES = 8
B, D, L = 16384, 1024, 4
B_LOC = B // N_CORES  # 2048 rows per core
P = 128               # SBUF partitions
N_TILES = B_LOC // P  # 16 batch tiles per core
N_DCH = D // P        # 8 contraction chunks of 128

F32 = mybir.dt.float32


def _build_program(
    zero_bias: bool, c_consts, reps: int = 1, hw_loop: int = 0, mode: str = "full"
):
    """Emit the per-core Bass program (SPMD: same NEFF on all 8 cores).

    reps > 1 repeats the whole computation back-to-back inside one NEFF;
    hw_loop = K > 0 wraps the body in a hardware For_i loop running K times.
    Both are used only for timing (slopes cancel dispatch overhead).

    mode (timing probes only; only "full" is numerically correct):
      "full"  - everything
      "not"   - no t-matmuls (transposes+copies kept, alpha memset)
      "notr"  - no transposes/copies (t-matmuls read un-transposed x, wrong)
      "scale" - no transposes/copies/t (alpha memset; DMA + scale only)
      "dma"   - DMA in + out only
    """
    nc = bacc.Bacc("TRN2", target_bir_lowering=False, debug=False)

    x_dram = nc.dram_tensor("x", [B_LOC, D], F32, kind="ExternalInput")
    wT_dram = nc.dram_tensor("wT", [P, N_DCH, L], F32, kind="ExternalInput")
    w_dram = (
        nc.dram_tensor("w", [L, D], F32, kind="ExternalInput")
        if mode.startswith("hyb")
        else None
    )
    if not zero_bias:
        yl_dram = nc.dram_tensor("yL", [1, D], F32, kind="ExternalInput")
    out_dram = nc.dram_tensor("out", [B_LOC, D], F32, kind="ExternalOutput")

    # hybN: tiles split between the PE path (transpose + matmul) and the DVE
    # path (fused multiply+reduce, no transpose needed) to balance engines.
    scale_act = mode.startswith("hyba")
    if mode.startswith("hyb"):
        n_pe = int(mode[4:] if scale_act else mode[3:])
        pe_tiles = {
            i
            for i in range(N_TILES)
            if (i * n_pe) // N_TILES != ((i + 1) * n_pe) // N_TILES
        }
    else:
        pe_tiles = set(range(N_TILES))

    big = mode == "fullsep"
    with tile.TileContext(nc) as tc:
        with (
            tc.tile_pool(name="consts", bufs=1) as consts,
            tc.tile_pool(name="xp", bufs=N_TILES + 1 if big else 6) as xp,
            tc.tile_pool(name="xtp", bufs=N_TILES + 1 if big else 4) as xtp,
            tc.tile_pool(name="outp", bufs=6) as outp,
            tc.tile_pool(name="small", bufs=12) as small,
            tc.tile_pool(name="ptr", bufs=5, space="PSUM") as ptr,
            tc.tile_pool(name="ptt", bufs=3, space="PSUM") as ptt,
            tc.tile_pool(name="scrp", bufs=2) as scrp,
        ):
            identity = consts.tile([P, P], F32)
            make_identity(nc, identity)

            wT = consts.tile([P, N_DCH, L], F32)
            nc.sync.dma_start(out=wT, in_=wT_dram[:])

            wb = None
            if mode.startswith("hyb"):
                # W rows replicated across partitions for the DVE ttr path.
                wb = consts.tile([P, L, D], F32)
                w_ap = w_dram[:]
                w_bcast = bass.AP(
                    tensor=w_ap.tensor,
                    offset=w_ap.offset,
                    ap=[[0, P], w_ap.ap[0], w_ap.ap[1]],
                )
                nc.sync.dma_start(out=wb, in_=w_bcast)

            if not zero_bias:
                # y_L row, replicated across all 128 partitions.
                ylb = consts.tile([P, D], F32)
                yl_ap = yl_dram[:]
                yl_bcast = bass.AP(
                    tensor=yl_ap.tensor,
                    offset=yl_ap.offset,
                    ap=[[0, P], yl_ap.ap[1]],
                )
                nc.sync.dma_start(out=ylb, in_=yl_bcast)

            def emit_a(i):
                """Phase A for tile i: DMA in, PE transposes, ACT copies.
                Returns (x_t, xT)."""
                x_t = xp.tile([P, D], F32)
                nc.sync.dma_start(out=x_t, in_=x_dram[i * P : (i + 1) * P, :])

                if mode == "dma":
                    nc.gpsimd.dma_start(
                        out=out_dram[i * P : (i + 1) * P, :], in_=x_t
                    )
                    return x_t, None

                xT = None
                if mode.startswith("hyb") and i in pe_tiles:
                    xT = xtp.tile([P, D], F32)
                    for g in range(2):
                        pt_tr = ptr.tile([P, 512], F32)
                        for jj in range(4):
                            j = g * 4 + jj
                            nc.tensor.transpose(
                                pt_tr[:, jj * P : (jj + 1) * P],
                                x_t[:, j * P : (j + 1) * P],
                                identity,
                            )
                        nc.scalar.copy(xT[:, g * 512 : (g + 1) * 512], pt_tr[:])
                    return x_t, xT
                if mode.startswith("hyb"):
                    return x_t, None
                if mode in ("full", "fullsep", "not", "noalpha", "noscale", "copydve", "tr2", "acc1"):
                    # Transpose the 8 [128,128] blocks of x_t on the PE so the
                    # contraction dim (d) lands on partitions; stage via PSUM.
                    xT = xtp.tile([P, D], F32)
                    for g in range(2):
                        pt_tr = ptr.tile([P, 512], F32)
                        for jj in range(4):
                            j = g * 4 + jj
                            nc.tensor.transpose(
                                pt_tr[:, jj * P : (jj + 1) * P],
                                x_t[:, j * P : (j + 1) * P],
                                identity,
                            )
                        # PSUM->SBUF staging copies on ACT (keeps DVE free for
                        # the alpha recurrence + final scale).
                        if mode == "copydve":
                            nc.vector.tensor_copy(
                                xT[:, g * 512 : (g + 1) * 512], pt_tr[:]
                            )
                        else:
                            nc.scalar.copy(xT[:, g * 512 : (g + 1) * 512], pt_tr[:])
                if mode == "tr2":
                    # 8 extra transpose-mode matmuls (same PE instruction count
                    # as full, but no normal-mode MMs / accumulation groups).
                    for g in range(2):
                        pt_x = ptr.tile([P, 512], F32, name="pt_x", tag="pt_tr")
                        for jj in range(4):
                            j = g * 4 + jj
                            nc.tensor.transpose(
                                pt_x[:, jj * P : (jj + 1) * P],
                                xT[:, j * P : (j + 1) * P],
                                identity,
                            )
                return x_t, xT

            def emit_b(i, x_t, xT):
                """Phase B for tile i: PE t-matmuls, DVE alpha+scale, DMA out."""
                t_ps = None
                t_wide = None
                t_sb = None
                if mode.startswith("hyb"):
                    if i in pe_tiles:
                        t_ps = ptt.tile([P, L], F32)
                        for j in range(N_DCH):
                            nc.tensor.matmul(
                                t_ps[:],
                                xT[:, j * P : (j + 1) * P],
                                wT[:, j, :],
                                start=(j == 0),
                                stop=(j == N_DCH - 1),
                            )
                    else:
                        # DVE path: fused multiply + free-axis reduce per layer
                        # via the production custom-DVE op (groupnorm_bwd uses
                        # this exact accum pattern on HW).
                        t_sb = small.tile([P, L], F32, name="tsb")
                        for l in range(L):
                            scr = scrp.tile([P, D], F32, name="scr")
                            nc.vector.affine_mul_reduce(
                                out=scr[:],
                                accum_out=t_sb[:, l : l + 1],
                                in0=x_t[:],
                                in1=wb[:, l, :],
                                scale=1.0,
                                bias=0.0,
                            )
                elif mode == "acc1":
                    # 8 independent start/stop matmuls into disjoint PSUM
                    # slices (no accumulation chain); summed on DVE.
                    src = xT
                    t_wide = ptt.tile([P, N_DCH, L], F32, name="tw")
                    for j in range(N_DCH):
                        nc.tensor.matmul(
                            t_wide[:, j, :],
                            src[:, j * P : (j + 1) * P],
                            wT[:, j, :],
                            start=True,
                            stop=True,
                        )
                elif mode in ("full", "fullsep", "notr", "noalpha", "noscale", "copydve"):
                    src = xT if xT is not None else x_t
                    # t tile [128 rows, 4 layers] accumulated over 8 d-chunks.
                    t_ps = ptt.tile([P, L], F32)
                    for j in range(N_DCH):
                        nc.tensor.matmul(
                            t_ps[:],
                            src[:, j * P : (j + 1) * P],
                            wT[:, j, :],
                            start=(j == 0),
                            stop=(j == N_DCH - 1),
                        )

                if t_wide is not None:
                    t_src = small.tile([P, L], F32, name="ts")
                    nc.vector.tensor_reduce(
                        t_src,
                        t_wide[:].rearrange("p j l -> p l j"),
                        axis=mybir.AxisListType.X,
                        op=mybir.AluOpType.add,
                    )
                    t_src = t_src[:]
                elif t_sb is not None:
                    t_src = t_sb[:]
                elif t_ps is not None:
                    t_src = t_ps[:]
                else:
                    t_src = None

                # alpha = prod_l (1 + t_l)  (+ c_l terms when biases != 0)
                alpha = small.tile([P, 1], F32)
                if t_src is None or mode == "noalpha":
                    nc.vector.memset(alpha, 1.0)
                elif zero_bias:
                    tp1 = small.tile([P, L], F32)
                    nc.vector.tensor_scalar_add(tp1, t_src, 1.0)
                    tm = small.tile([P, 2], F32)
                    nc.vector.tensor_tensor(
                        tm, tp1[:, 0:2], tp1[:, 2:4], mybir.AluOpType.mult
                    )
                    nc.vector.tensor_tensor(
                        alpha, tm[:, 0:1], tm[:, 1:2], mybir.AluOpType.mult
                    )
                else:
                    a_cur = None
                    for l in range(L):
                        anew = small.tile([P, 1], F32, name=f"a{l}_{i}")
                        if a_cur is None:
                            nc.vector.tensor_scalar_add(
                                anew, t_src[:, l : l + 1], 1.0 + float(c_consts[l])
                            )
                        else:
                            nc.vector.tensor_scalar(
                                anew,
                                t_src[:, l : l + 1],
                                1.0,
                                a_cur[:],
                                mybir.AluOpType.add,
                                mybir.AluOpType.mult,
                            )
                            if float(c_consts[l]) != 0.0:
                                nc.vector.tensor_scalar_add(
                                    anew, anew[:], float(c_consts[l])
                                )
                        a_cur = anew
                    nc.vector.tensor_copy(alpha, a_cur[:])

                # out = x0 * alpha (+ y_L)
                o_t = outp.tile([P, D], F32)
                if zero_bias:
                    if scale_act:
                        # ACT scale; Identity func (Copy with an AP scale
                        # crashes the exec unit on this runtime).
                        nc.scalar.activation(
                            o_t,
                            x_t[:],
                            mybir.ActivationFunctionType.Identity,
                            bias=0.0,
                            scale=alpha[:],
                        )
                    else:
                        nc.vector.tensor_scalar_mul(o_t, x_t[:], alpha[:])
                else:
                    nc.vector.scalar_tensor_tensor(
                        o_t,
                        x_t[:],
                        alpha[:],
                        ylb[:],
                        mybir.AluOpType.mult,
                        mybir.AluOpType.add,
                    )
                # Output DMAs triggered from gpsimd so their dispatch doesn't
                # serialize behind the input DMAs on the sync sequencer.
                nc.gpsimd.dma_start(out=out_dram[i * P : (i + 1) * P, :], in_=o_t)

            def emit_all():
                # Software-pipelined emission with a 1-tile skew: tile i's
                # t-matmuls are emitted after tile i+1's transposes so the PE
                # stream never waits on the ACT staging copies.
                if mode == "dma":
                    for i in range(N_TILES):
                        emit_a(i)
                    return
                if mode == "fullsep":
                    # Two separated passes: all transpose-mode PE work first,
                    # then all normal-mode matmuls — a single PE mode switch
                    # (each transpose<->matmul transition costs a PE stall).
                    staged = [emit_a(i) for i in range(N_TILES)]
                    for i in range(N_TILES):
                        emit_b(i, *staged[i])
                    return
                if mode.startswith("hyb"):
                    # PE tiles need the skewed two-phase emission; DVE tiles
                    # are self-contained. Interleave so both engines fill.
                    staged = {}
                    for i in range(N_TILES + 1):
                        if i < N_TILES:
                            staged[i] = emit_a(i)
                            if i not in pe_tiles:
                                x_t, xT = staged.pop(i)
                                emit_b(i, x_t, xT)
                        if i >= 1 and (i - 1) in staged:
                            x_t, xT = staged.pop(i - 1)
                            emit_b(i - 1, x_t, xT)
                    return
                staged = {}
                for i in range(N_TILES + 1):
                    if i < N_TILES:
                        staged[i] = emit_a(i)
                    if i >= 1:
                        x_t, xT = staged.pop(i - 1)
                        emit_b(i - 1, x_t, xT)

            if hw_loop > 0:
                with tc.For_i(
                    0, hw_loop, 1, hint_engines=(mybir.EngineType.PE,)
                ) as _iv:
                    emit_all()
            else:
                for _rep in range(reps):
                    emit_all()

    nc.compile()
    return nc


_CACHE = {}


# Best measured configuration: 8 tiles on the PE (transpose+matmul) path,
# 8 on the DVE (affine_mul_reduce) path, final scale on ACT.
BEST_MODE = "hyba8"


def _get_program(zero_bias: bool, c_key):
    key = (zero_bias, c_key)
    if key not in _CACHE:
        c_consts = list(c_key) if c_key is not None else None
        _CACHE[key] = _build_program(zero_bias, c_consts, mode=BEST_MODE)
    return _CACHE[key]


def kernel(inputs, weights, biases, _trace=False, _bass_results=None):
    inputs = np.ascontiguousarray(np.asarray(inputs, dtype=np.float32))
    weights = np.ascontiguousarray(np.asarray(weights, dtype=np.float32))
    biases = np.ascontiguousarray(np.asarray(biases, dtype=np.float32))
    assert inputs.shape == (B, D) and weights.shape == (L, D) and biases.shape == (L, D)

    zero_bias = bool(np.all(biases == 0.0))

    # Host-side prep of the tiny replicated weight tensors.
    # wT[p, j, l] = W[l, j*128 + p]  (W^T in d-chunked, partition-major layout)
    wT_np = np.ascontiguousarray(
        weights.T.reshape(N_DCH, P, L).transpose(1, 0, 2)
    ).astype(np.float32)

    if zero_bias:
        c_key = None
        yl_np = None
    else:
        # y_l = sum_{j<l} b_j ;  c_l = y_l . w_l
        y = np.zeros(D, dtype=np.float64)
        c = []
        for l in range(L):
            c.append(float(np.dot(y, weights[l].astype(np.float64))))
            y = y + biases[l].astype(np.float64)
        c_key = tuple(c)
        yl_np = np.ascontiguousarray(y.astype(np.float32).reshape(1, D))

    nc = _get_program(zero_bias, c_key)

    in_maps = []
    for core in range(N_CORES):
        m = {
            "x": inputs[core * B_LOC : (core + 1) * B_LOC],
            "wT": wT_np,
        }
        if BEST_MODE.startswith("hyb"):
            m["w"] = weights
        if not zero_bias:
            m["yL"] = yl_np
        in_maps.append(m)

    res = run_bass_kernel_spmd(
        nc, in_maps, core_ids=list(range(N_CORES)), trace=_trace
    )
    if _bass_results is not None:
        _bass_results.append(res)

    out = np.concatenate([res.results[c]["out"] for c in range(N_CORES)], axis=0)
    return out
